# revision 6
# baseline (speedup 1.0000x reference)
"""Expert-parallel grouped MLP (MoE routing) for Trainium2.

Problem: x[16384,1024] fp32, w1[8,1024,4096], w2[8,4096,1024],
rows_per_expert=2048.  out = gelu(x_e @ w1[e]) @ w2[e] per expert group.

Sharding: one expert per NeuronCore (E=8 == n_cores).  Each core runs an
identical Bass program on its own expert's slice; no collectives.  The host
pre-permutes each operand so every DMA chunk is a fully contiguous DRAM
region with 2-8KB per-partition lines:
    x  -> [NBLK, 128, HO, T_BLK]   (xp[b,p,h,ti]  = x[b*T_BLK+ti, h*128+p])
    w1 -> [FO, 128, H]             (w1p[f,p,h*128+fi] = w1[h*128+p, f*128+fi])
    w2 -> [HO, 128, F]             (w2p[h,p,f*128+hi] = w2[f*128+p, h*128+hi])
    out <- [NBLK, HO, 128, T_BLK]  (out4[b,h,p,ti] = out[b*T_BLK+ti, h*128+p])
Activations stay in [feature, token] orientation through both GEMMs:
    GEMM1: interT[f,t] = sum_h w1[h,f] * xT[h,t]    (lhsT = w1 tile)
    gelu on PSUM -> SBUF (bf16)
    GEMM2: outT[h,t]  = sum_f w2[f,h] * interT[f,t]  (lhsT = w2 tile)
Matmuls run in bf16 (fp32 PSUM accumulate) - fp32 matmul is 4x slower on
the PE array.  Weights are SBUF-resident (64KB/partition each); tokens are
processed in 4 blocks of 512 so interT fits in SBUF.  Weight-chunk DMAs are
gated on compute progress (only ~1.3MB gates the first matmul) and dummy
matmuls on a zeroed tile warm the PE clock (HAM) during the initial DMA.
"""

import numpy as np
import ml_dtypes

E = 8
H = 1024
F = 4096
T_PER_E = 2048
T_BLK = 512
NBLK = T_PER_E // T_BLK
P = 128
HO = H // P    # 8 contraction chunks for GEMM1
FO = F // P    # 32 contraction chunks for GEMM2
NW2 = 8        # w2 staged in HO chunks
NWARM = 20     # PE warm-up matmuls (N=128: cover until startup DMAs land)
W1_UNGATED = 5     # leading w1 chunks that stream immediately
W1_LOOKAHEAD = 5   # f-tiles of slack between a w1 chunk's DMA gate and its use

TRACE = False          # test.py sets kernel.TRACE = True for profiling
LAST_RESULTS = None    # BassKernelResults of the most recent run

_nc_cache = None


def _build_nc():
    import concourse.mybir as mybir
    import concourse.tile as tile
    from concourse import bacc
    from concourse.tile_rust import add_dep_helper

    bf16 = mybir.dt.bfloat16
    f32 = mybir.dt.float32
    GELU = mybir.ActivationFunctionType.Gelu_apprx_tanh

    nc = bacc.Bacc("TRN2", target_bir_lowering=False, debug=False)

    xp = nc.dram_tensor("xp", [NBLK, P, HO, T_BLK], bf16, kind="ExternalInput").ap()
    w1p = nc.dram_tensor("w1p", [FO, P, H], bf16, kind="ExternalInput").ap()
    w2p = nc.dram_tensor("w2p", [HO, P, F], bf16, kind="ExternalInput").ap()
    # Output in bf16: halves the store traffic draining at the kernel tail;
    # the host upcasts to fp32.  The added rounding (~1e-3 relative, on top
    # of the ~3.4e-3 from the bf16 matmuls) is negligible.
    out4 = nc.dram_tensor("out4", [NBLK, HO, P, T_BLK], bf16, kind="ExternalOutput").ap()

    with tile.TileContext(nc) as tc:
        with (
            tc.tile_pool(name="wpool", bufs=1) as wpool,
            tc.tile_pool(name="xpool", bufs=2) as xpool,
            tc.tile_pool(name="ipool", bufs=1) as ipool,
            tc.tile_pool(name="opool", bufs=3) as opool,
            tc.tile_pool(name="ps1", bufs=4, space="PSUM") as ps1,
            tc.tile_pool(name="ps2", bufs=4, space="PSUM") as ps2,
        ):
            # PE warm-up: dummy N=128 matmuls on a small zeroed tile keep the
            # PE busy while the first real operands stream in, so the HAM
            # clock gate reaches full rate before the first real matmul.  The
            # [128,128] memset is ~4x cheaper than a full [128,512] tile, so
            # the warm stream starts earlier.
            warm = wpool.tile([P, P], bf16, tag="warm")
            nc.any.memset(warm[:], 0.0)
            for _ in range(NWARM):
                wp = ps1.tile([P, T_BLK], f32, tag="ps1t")
                nc.tensor.matmul(wp[:, 0:P], warm[:], warm[:], start=True, stop=True)

            # w1 layout [P, FO, H]: lhsT for (h,f) = w1_sb[:, f, h*128:(h+1)*128]
            # w2 layout [P, HO, F]: lhsT for (f,h) = w2_sb[:, h, f*128:(f+1)*128]
            w1_sb = wpool.tile([P, FO, H], bf16, tag="w1sb")
            w2_sb = wpool.tile([P, HO, F], bf16, tag="w2sb")

            # Startup is co-limited by trigger issue (~0.65us of sequencer
            # time per HWDGE trigger) and DMA bandwidth (~366 GB/s shared by
            # all active queues).  SP streams xb0 as h-chunk PAIRS (256KB per
            # trigger, so copy time ~ issue time and the chunks land in
            # need-order at full bandwidth); the Activation sequencer (idle
            # until the first gelu) issues w1[0..2] in parallel so the first
            # f-chains' weights overlap the x stream.
            xb0 = xpool.tile([P, HO, T_BLK], bf16, tag="xb")
            w1_dmas = [None] * FO
            xb0_dmas = []
            xb0_dmas.append(nc.sync.dma_start(xb0[:, 0:2, :], xp[0, :, 0:2, :]))
            for c in range(3):
                w1_dmas[c] = nc.scalar.dma_start(w1_sb[:, c, :], w1p[c])
            for h in range(2, HO, 2):
                xb0_dmas.append(
                    nc.sync.dma_start(xb0[:, h:h + 2, :], xp[0, :, h:h + 2, :])
                )
            for f in range(3, FO):
                w1_dmas[f] = nc.sync.dma_start(w1_sb[:, f, :], w1p[f])
            w2_dmas = [
                nc.sync.dma_start(w2_sb[:, h, :], w2p[h]) for h in range(NW2)
            ]
            mm_first = {}  # (b, f) -> first matmul of that f-tile

            for b in range(NBLK):
                if b == 0:
                    xb = xb0
                else:
                    xb = xpool.tile([P, HO, T_BLK], bf16, tag="xb")
                    nc.sync.dma_start(xb[:], xp[b])

                it = ipool.tile([P, FO, T_BLK], bf16, tag="inter")
                for f in range(FO):
                    ps = ps1.tile([P, T_BLK], f32, tag="ps1t")
                    for h in range(HO):
                        mm = nc.tensor.matmul(
                            ps[:],
                            w1_sb[:, f, h * P:(h + 1) * P],
                            xb[:, h, :],
                            start=(h == 0),
                            stop=(h == HO - 1),
                        )
                        if h == 0:
                            mm_first[(b, f)] = mm
                    nc.scalar.activation(it[:, f, :], ps[:], GELU)

                HB = T_BLK // 2
                for h in range(HO):
                    if b == NBLK - 1 and h == HO - 1:
                        # Final tile: run two column-half chains so the first
                        # half's evict+store overlaps the second half's
                        # matmuls, and only a half-width evict+store remains
                        # after the last matmul.
                        ob = opool.tile([P, T_BLK], bf16, tag="ob")
                        for half in range(2):
                            cols = slice(half * HB, (half + 1) * HB)
                            ps = ps2.tile([P, T_BLK], f32, tag="ps2t")
                            for f in range(FO):
                                nc.tensor.matmul(
                                    ps[:, 0:HB],
                                    w2_sb[:, h, f * P:(f + 1) * P],
                                    it[:, f, cols],
                                    start=(f == 0),
                                    stop=(f == FO - 1),
                                )
                            nc.vector.tensor_copy(ob[:, cols], ps[:, 0:HB])
                            nc.sync.dma_start(out4[b, h, :, cols], ob[:, cols])
                        continue
                    ps = ps2.tile([P, T_BLK], f32, tag="ps2t")
                    for f in range(FO):
                        nc.tensor.matmul(
                            ps[:],
                            w2_sb[:, h, f * P:(f + 1) * P],
                            it[:, f, :],
                            start=(f == 0),
                            stop=(f == FO - 1),
                        )
                    # Evict in two halves so the DMA store of the first half
                    # overlaps the copy of the second (shortens the kernel
                    # tail after the last matmul).
                    ob = opool.tile([P, T_BLK], bf16, tag="ob")
                    nc.vector.tensor_copy(ob[:, :HB], ps[:, :HB])
                    nc.sync.dma_start(out4[b, h, :, :HB], ob[:, :HB])
                    nc.vector.tensor_copy(ob[:, HB:], ps[:, HB:])
                    nc.sync.dma_start(out4[b, h, :, HB:], ob[:, HB:])

            # Stage the weight stream behind compute progress so the bulk of
            # the 16MB of weights never contends with the critical path:
            # w1 f-tile chunk c waits for the f-tile W1_LOOKAHEAD tiles ahead
            # of its first consumer; w2 chunk c is gated on the tail f-tiles
            # of GEMM1 block 0 (w2 is first read ~55us in).
            for c in range(W1_UNGATED, FO):
                add_dep_helper(
                    w1_dmas[c].ins, mm_first[(0, c - W1_LOOKAHEAD)].ins,
                    sync=True, reason="stage w1 load behind compute",
                )
            for c in range(NW2):
                add_dep_helper(
                    w2_dmas[c].ins, mm_first[(0, FO - NW2 - 6 + c)].ins,
                    sync=True, reason="stage w2 load behind compute",
                )
    nc.compile()
    return nc


def _get_nc():
    global _nc_cache
    if _nc_cache is None:
        _nc_cache = _build_nc()
    return _nc_cache


def kernel(x, w1, w2, rows_per_expert):
    global LAST_RESULTS
    from concourse.bass_utils import run_bass_kernel_spmd

    x = np.asarray(x)
    w1 = np.asarray(w1)
    w2 = np.asarray(w2)
    rpe = int(rows_per_expert)
    assert x.shape == (E * rpe, H) and rpe == T_PER_E
    assert w1.shape == (E, H, F) and w2.shape == (E, F, H)

    bf16 = ml_dtypes.bfloat16
    in_maps = []
    for e in range(E):
        xe = x[e * rpe:(e + 1) * rpe].astype(bf16)      # [T, H]
        # [b*T_BLK+ti, ho*128+p] -> [b, p, ho, ti]
        xpm = np.ascontiguousarray(
            xe.reshape(NBLK, T_BLK, HO, P).transpose(0, 3, 2, 1)
        )
        # w1[ho*128+p, f*128+fi] -> [f, p, ho*128+fi]
        w1m = np.ascontiguousarray(
            w1[e].astype(bf16).reshape(HO, P, FO, P).transpose(2, 1, 0, 3)
        ).reshape(FO, P, H)
        # w2[fo*128+p, h*128+hi] -> [h, p, fo*128+hi]
        w2m = np.ascontiguousarray(
            w2[e].astype(bf16).reshape(FO, P, HO, P).transpose(2, 1, 0, 3)
        ).reshape(HO, P, F)
        in_maps.append({"xp": xpm, "w1p": w1m, "w2p": w2m})

    res = run_bass_kernel_spmd(_get_nc(), in_maps, list(range(E)), trace=TRACE)
    LAST_RESULTS = res

    out = np.empty((E * rpe, H), dtype=np.float32)
    for e in range(E):
        # [b, h, p, ti] -> [b*T_BLK+ti, h*128+p]
        o4 = res.results[e]["out4"].astype(np.float32)
        out[e * rpe:(e + 1) * rpe] = o4.transpose(0, 3, 1, 2).reshape(rpe, H)
    return out



# revision 8
# speedup vs baseline: 1.0006x; 1.0006x over previous
"""Expert-parallel grouped MLP (MoE routing) for Trainium2.

Problem: x[16384,1024] fp32, w1[8,1024,4096], w2[8,4096,1024],
rows_per_expert=2048.  out = gelu(x_e @ w1[e]) @ w2[e] per expert group.

Sharding: one expert per NeuronCore (E=8 == n_cores).  Each core runs an
identical Bass program on its own expert's slice; no collectives.  The host
pre-permutes each operand so every DMA chunk is a fully contiguous DRAM
region with 2-8KB per-partition lines:
    x  -> [NBLK, 128, HO, T_BLK]   (xp[b,p,h,ti]  = x[b*T_BLK+ti, h*128+p])
    w1 -> [FO, 128, H]             (w1p[f,p,h*128+fi] = w1[h*128+p, f*128+fi])
    w2 -> [HO, 128, F]             (w2p[h,p,f*128+hi] = w2[f*128+p, h*128+hi])
    out <- [NBLK, HO, 128, T_BLK]  (out4[b,h,p,ti] = out[b*T_BLK+ti, h*128+p])
Activations stay in [feature, token] orientation through both GEMMs:
    GEMM1: interT[f,t] = sum_h w1[h,f] * xT[h,t]    (lhsT = w1 tile)
    gelu on PSUM -> SBUF (bf16)
    GEMM2: outT[h,t]  = sum_f w2[f,h] * interT[f,t]  (lhsT = w2 tile)
Matmuls run in bf16 (fp32 PSUM accumulate) - fp32 matmul is 4x slower on
the PE array.  Weights are SBUF-resident (64KB/partition each); tokens are
processed in 4 blocks of 512 so interT fits in SBUF.  Weight-chunk DMAs are
gated on compute progress (only ~1.3MB gates the first matmul) and dummy
matmuls on a zeroed tile warm the PE clock (HAM) during the initial DMA.
"""

import numpy as np
import ml_dtypes

E = 8
H = 1024
F = 4096
T_PER_E = 2048
T_BLK = 512
NBLK = T_PER_E // T_BLK
P = 128
HO = H // P    # 8 contraction chunks for GEMM1
FO = F // P    # 32 contraction chunks for GEMM2
NW2 = 8        # w2 staged in HO chunks
NWARM = 26     # PE warm-up matmuls (N=128: cover until startup DMAs land)
W1_UNGATED = 5     # leading w1 chunks that stream immediately
W1_LOOKAHEAD = 5   # f-tiles of slack between a w1 chunk's DMA gate and its use

TRACE = False          # test.py sets kernel.TRACE = True for profiling
LAST_RESULTS = None    # BassKernelResults of the most recent run

_nc_cache = None


def _build_nc():
    import concourse.mybir as mybir
    import concourse.tile as tile
    from concourse import bacc
    from concourse.tile_rust import add_dep_helper

    bf16 = mybir.dt.bfloat16
    f32 = mybir.dt.float32
    GELU = mybir.ActivationFunctionType.Gelu_apprx_tanh

    nc = bacc.Bacc("TRN2", target_bir_lowering=False, debug=False)

    xp = nc.dram_tensor("xp", [NBLK, P, HO, T_BLK], bf16, kind="ExternalInput").ap()
    w1p = nc.dram_tensor("w1p", [FO, P, H], bf16, kind="ExternalInput").ap()
    w2p = nc.dram_tensor("w2p", [HO, P, F], bf16, kind="ExternalInput").ap()
    # Output in bf16: halves the store traffic draining at the kernel tail;
    # the host upcasts to fp32.  The added rounding (~1e-3 relative, on top
    # of the ~3.4e-3 from the bf16 matmuls) is negligible.
    out4 = nc.dram_tensor("out4", [NBLK, HO, P, T_BLK], bf16, kind="ExternalOutput").ap()

    with tile.TileContext(nc) as tc:
        with (
            tc.tile_pool(name="wpool", bufs=1) as wpool,
            tc.tile_pool(name="xpool", bufs=2) as xpool,
            tc.tile_pool(name="ipool", bufs=1) as ipool,
            tc.tile_pool(name="opool", bufs=3) as opool,
            tc.tile_pool(name="ps1", bufs=4, space="PSUM") as ps1,
            tc.tile_pool(name="ps2", bufs=4, space="PSUM") as ps2,
        ):
            # PE warm-up: dummy N=128 matmuls on a small zeroed tile keep the
            # PE busy while the first real operands stream in, so the HAM
            # clock gate reaches full rate before the first real matmul.  The
            # [128,128] memset is ~4x cheaper than a full [128,512] tile, so
            # the warm stream starts earlier.
            warm = wpool.tile([P, P], bf16, tag="warm")
            nc.any.memset(warm[:], 0.0)
            for _ in range(NWARM):
                wp = ps1.tile([P, T_BLK], f32, tag="ps1t")
                nc.tensor.matmul(wp[:, 0:P], warm[:], warm[:], start=True, stop=True)

            # w1 layout [P, FO, H]: lhsT for (h,f) = w1_sb[:, f, h*128:(h+1)*128]
            # w2 layout [P, HO, F]: lhsT for (f,h) = w2_sb[:, h, f*128:(f+1)*128]
            w1_sb = wpool.tile([P, FO, H], bf16, tag="w1sb")
            w2_sb = wpool.tile([P, HO, F], bf16, tag="w2sb")

            # Each engine's HWDGE triggers land on ONE queue which executes
            # its copies serially in trigger order; concurrently-active
            # queues SPLIT the ~330 B/ns DMA bandwidth.  The startup is
            # bandwidth-bound (1.25MB must land before the f1 chain), so the
            # optimal schedule is a single queue in exact need-order at full
            # bandwidth: w1[0], then xb0 as h-chunk pairs, then w1[1..4].
            # Scalar issues nothing at startup so queue 1 runs uncontended.
            xb0 = xpool.tile([P, HO, T_BLK], bf16, tag="xb")
            w1_dmas = [None] * FO
            w1_dmas[0] = nc.sync.dma_start(w1_sb[:, 0, :], w1p[0])
            xb0_dmas = [
                nc.sync.dma_start(xb0[:, h:h + 2, :], xp[0, :, h:h + 2, :])
                for h in range(0, HO, 2)
            ]
            for f in range(1, FO):
                w1_dmas[f] = nc.sync.dma_start(w1_sb[:, f, :], w1p[f])
            w2_dmas = [
                nc.sync.dma_start(w2_sb[:, h, :], w2p[h]) for h in range(NW2)
            ]
            mm_first = {}  # (b, f) -> first matmul of that f-tile

            for b in range(NBLK):
                if b == 0:
                    xb = xb0
                else:
                    xb = xpool.tile([P, HO, T_BLK], bf16, tag="xb")
                    nc.sync.dma_start(xb[:], xp[b])

                it = ipool.tile([P, FO, T_BLK], bf16, tag="inter")
                for f in range(FO):
                    ps = ps1.tile([P, T_BLK], f32, tag="ps1t")
                    for h in range(HO):
                        mm = nc.tensor.matmul(
                            ps[:],
                            w1_sb[:, f, h * P:(h + 1) * P],
                            xb[:, h, :],
                            start=(h == 0),
                            stop=(h == HO - 1),
                        )
                        if h == 0:
                            mm_first[(b, f)] = mm
                    nc.scalar.activation(it[:, f, :], ps[:], GELU)

                HB = T_BLK // 2
                for h in range(HO):
                    if b == NBLK - 1 and h == HO - 1:
                        # Final tile: run two column-half chains so the first
                        # half's evict+store overlaps the second half's
                        # matmuls, and only a half-width evict+store remains
                        # after the last matmul.
                        ob = opool.tile([P, T_BLK], bf16, tag="ob")
                        for half in range(2):
                            cols = slice(half * HB, (half + 1) * HB)
                            ps = ps2.tile([P, T_BLK], f32, tag="ps2t")
                            for f in range(FO):
                                nc.tensor.matmul(
                                    ps[:, 0:HB],
                                    w2_sb[:, h, f * P:(f + 1) * P],
                                    it[:, f, cols],
                                    start=(f == 0),
                                    stop=(f == FO - 1),
                                )
                            nc.vector.tensor_copy(ob[:, cols], ps[:, 0:HB])
                            nc.sync.dma_start(out4[b, h, :, cols], ob[:, cols])
                        continue
                    ps = ps2.tile([P, T_BLK], f32, tag="ps2t")
                    for f in range(FO):
                        nc.tensor.matmul(
                            ps[:],
                            w2_sb[:, h, f * P:(f + 1) * P],
                            it[:, f, :],
                            start=(f == 0),
                            stop=(f == FO - 1),
                        )
                    # Evict in two halves so the DMA store of the first half
                    # overlaps the copy of the second (shortens the kernel
                    # tail after the last matmul).
                    ob = opool.tile([P, T_BLK], bf16, tag="ob")
                    nc.vector.tensor_copy(ob[:, :HB], ps[:, :HB])
                    nc.sync.dma_start(out4[b, h, :, :HB], ob[:, :HB])
                    nc.vector.tensor_copy(ob[:, HB:], ps[:, HB:])
                    nc.sync.dma_start(out4[b, h, :, HB:], ob[:, HB:])

            # Stage the weight stream behind compute progress so the bulk of
            # the 16MB of weights never contends with the critical path:
            # w1 f-tile chunk c waits for the f-tile W1_LOOKAHEAD tiles ahead
            # of its first consumer; w2 chunk c is gated on the tail f-tiles
            # of GEMM1 block 0 (w2 is first read ~55us in).
            for c in range(W1_UNGATED, FO):
                add_dep_helper(
                    w1_dmas[c].ins, mm_first[(0, c - W1_LOOKAHEAD)].ins,
                    sync=True, reason="stage w1 load behind compute",
                )
            for c in range(NW2):
                add_dep_helper(
                    w2_dmas[c].ins, mm_first[(0, FO - NW2 - 6 + c)].ins,
                    sync=True, reason="stage w2 load behind compute",
                )
    nc.compile()
    return nc


def _get_nc():
    global _nc_cache
    if _nc_cache is None:
        _nc_cache = _build_nc()
    return _nc_cache


def kernel(x, w1, w2, rows_per_expert):
    global LAST_RESULTS
    from concourse.bass_utils import run_bass_kernel_spmd

    x = np.asarray(x)
    w1 = np.asarray(w1)
    w2 = np.asarray(w2)
    rpe = int(rows_per_expert)
    assert x.shape == (E * rpe, H) and rpe == T_PER_E
    assert w1.shape == (E, H, F) and w2.shape == (E, F, H)

    bf16 = ml_dtypes.bfloat16
    in_maps = []
    for e in range(E):
        xe = x[e * rpe:(e + 1) * rpe].astype(bf16)      # [T, H]
        # [b*T_BLK+ti, ho*128+p] -> [b, p, ho, ti]
        xpm = np.ascontiguousarray(
            xe.reshape(NBLK, T_BLK, HO, P).transpose(0, 3, 2, 1)
        )
        # w1[ho*128+p, f*128+fi] -> [f, p, ho*128+fi]
        w1m = np.ascontiguousarray(
            w1[e].astype(bf16).reshape(HO, P, FO, P).transpose(2, 1, 0, 3)
        ).reshape(FO, P, H)
        # w2[fo*128+p, h*128+hi] -> [h, p, fo*128+hi]
        w2m = np.ascontiguousarray(
            w2[e].astype(bf16).reshape(FO, P, HO, P).transpose(2, 1, 0, 3)
        ).reshape(HO, P, F)
        in_maps.append({"xp": xpm, "w1p": w1m, "w2p": w2m})

    res = run_bass_kernel_spmd(_get_nc(), in_maps, list(range(E)), trace=TRACE)
    LAST_RESULTS = res

    out = np.empty((E * rpe, H), dtype=np.float32)
    for e in range(E):
        # [b, h, p, ti] -> [b*T_BLK+ti, h*128+p]
        o4 = res.results[e]["out4"].astype(np.float32)
        out[e * rpe:(e + 1) * rpe] = o4.transpose(0, 3, 1, 2).reshape(rpe, H)
    return out

